# revision 21
# baseline (speedup 1.0000x reference)
"""AdaArcFace loss on 8 TRN2 NeuronCores (Bass, class-sharded tensor parallel).

loss = mean_i( LSE_i - 32*cosm_i ),  LSE_i = 32 + ln(S_i + em_i - ey_i)
  S_i  = sum_c exp(32*(cos[i,c] - 1))   <- the only term needing the big matmul
  cos_y/quantile/margin path is tiny, exact fp32, replicated on every core.

Sharding: 100000 classes -> 8 cores x 12544 (44 pad rows = -features[0], whose
softmax contribution is ~1e-17 relative). kernel shard is cast to bf16 and
transposed on host (pack only) so DMA streams contiguous at half the bytes and
the PE gets emb-on-partitions.

v4 (from v3's 223-278us baseline; trace showed PE half-clocked by HAM, ACT
square pass + SWDGE flatten dominating):
 - weights stream as bf16: DMA 72->37us, FWL on the bf16 stationary.
 - norm^2 via ones-stationary broadcast matmul accumulating the 4 emb chunks
   of W2 straight into PSUM q-slots: kills the ACT square pass, DVE pairsums,
   the 98 one-row qmm matmuls, and the 7392-packet SWDGE flatten/broadcast.
 - inv = exp(-.5 ln q + ln32) on [128,256] q-chunks; everything ACT ever runs
   (Square/Ln/Exp/Copy) lives in the one natural_log_exp table set. cos(m*pi)
   moved to a DVE sin-polynomial so the trig table never loads.
 - PE is the only saturated engine -> HAM stays at 8/8 after warmup.
 - squares on DVE bf16 (2x mode), v_mult per phase off PSUM, exp+accum on ACT.
"""

import math
import numpy as np

import concourse.bass as bass
import concourse.mybir as mybir
from concourse import library_config
from concourse.bass_utils import run_bass_kernel_spmd

F32 = mybir.dt.float32
BF16 = mybir.dt.bfloat16

# problem constants (hardcoded per harness contract)
B = 256          # batch
E = 512          # embedding
C = 100000       # classes
NCORES = 8
CPC = 12544      # classes per core (padded): 98 groups of 128
TILE_C = 1792    # classes per stream tile
NTILES = CPC // TILE_C             # 7
NPH = 2 * NTILES                   # 14 phases (bc-halves)
QW = 256                           # q broadcast chunk width
NQC = TILE_C // QW                 # 7 q-chunks per tile
CHUNKS = [(0, 512), (512, 512), (1024, 512), (1536, 256)]  # D chunk (off, w)
SCALE = 32.0
M_BASE = 0.5
ALPHA = 0.1
BETA = 0.15
SIN_M = math.sin(M_BASE)
LN32 = math.log(32.0)
ECH = E // 128   # 4 emb chunks

_CACHE = {}


def build_nc():
    nc = bass.Bass(target_bir_lowering=False, num_devices=NCORES, num_swdge_queues=2)

    wt_ext = nc.declare_dram_parameter(
        "wt", [NTILES, ECH, 128, TILE_C], BF16, isOutput=False)
    feat_ext = nc.declare_dram_parameter("feat", [B, E], F32, isOutput=False)
    wlab_ext = nc.declare_dram_parameter("wlab", [B, E], F32, isOutput=False)
    cpack_ext = nc.declare_dram_parameter("cpack", [128, 132], F32, isOutput=False)
    cbf_ext = nc.declare_dram_parameter("cbf", [128, 128], BF16, isOutput=False)
    out_ext = nc.declare_dram_parameter("out", [1, 1], F32, isOutput=True)

    # dummy collective: forces the runtime's synchronized multi-core launch
    cc_in = nc.dram_tensor("cc_in", [1, 1], F32)
    cc_out = nc.dram_tensor("cc_out", [1, NCORES], F32, addr_space="Shared")

    WT_TILE_ELEMS = ECH * 128 * TILE_C

    from contextlib import ExitStack
    ctx = ExitStack()
    sb = lambda name, shape, dt=F32: ctx.enter_context(nc.sbuf_tensor(name, shape, dt))
    ps = lambda name, shape, dt=F32: ctx.enter_context(nc.psum_tensor(name, shape, dt))
    sem = lambda name: ctx.enter_context(nc.semaphore(name))

    with ctx:
        # --- SBUF ---
        WT = [sb(f"WT{i}", [128, ECH, TILE_C], BF16) for i in range(3)]
        W2 = [sb(f"W2{i}", [128, ECH, TILE_C], BF16) for i in range(2)]
        LNB = [sb(f"LNB{i}", [128, TILE_C]) for i in range(2)]
        INVB = [sb(f"INVB{i}", [128, TILE_C]) for i in range(2)]  # 32/||w||
        CN = [sb(f"CN{i}", [128, TILE_C], BF16) for i in range(2)]  # 32*cos
        EJ = [sb(f"EJ{i}", [128, TILE_C], BF16) for i in range(2)]  # exp junk
        SACC = sb("SACC", [128, 2, NTILES])                     # accum_out slots
        FT = sb("FT", [128, ECH, 2, 128], BF16)                 # fhatT bf16
        F_ = sb("F", [128, 2, E])
        FN = sb("FN", [128, 2, E])
        WL = sb("WL", [128, 2, E])
        WLN = sb("WLN", [128, 2, E])
        SCR = sb("SCR", [128, E])      # square/cosy scratch
        CMP2 = sb("CMP2", [128, B])
        DB = sb("DB", [128, B])        # difficulty broadcast (rank compares)
        DDR = sb("DDR", [1, B])        # difficulty flattened row
        AG = sb("AG", [128, 2 * NCORES])  # remote slot j at cols [2j, 2j+2)
        qfw = sb("qfw", [128, 4])
        lnq4 = sb("lnq4", [128, 4])
        invfw = sb("invfw", [128, 4])
        cosy = sb("cosy", [128, 2]); dd = sb("dd", [128, 2])
        cnt = sb("cnt", [128, 2]); mask = sb("mask", [128, 2])
        t1 = sb("t1", [128, 2]); m015 = sb("m015", [128, 2]); mm_ = sb("mm", [128, 2])
        xs = sb("xs", [128, 2]); x2 = sb("x2", [128, 2])
        pa = sb("pa", [128, 2]); pb = sb("pb", [128, 2]); pc = sb("pc", [128, 2])
        cmpv = sb("cmpv", [128, 2]); t2 = sb("t2", [128, 2]); t3 = sb("t3", [128, 2])
        cosm = sb("cosm", [128, 2]); ey = sb("ey", [128, 2]); em = sb("em", [128, 2])
        adj = sb("adj", [128, 2]); li0c = sb("li0c", [128, 2])
        Sb = sb("Sb", [128, 2]); Sf1 = sb("Sf1", [128, 2])
        Sfull = sb("Sfull", [128, 2]); TT = sb("TT", [128, 2])
        lS = sb("lS", [128, 2]); li = sb("li", [128, 2])
        lsum = sb("lsum", [1, 1]); loss = sb("loss", [1, 1])
        cpack = sb("cpk", [128, 132])
        cbf = sb("cbfs", [128, 128], BF16)

        # --- PSUM: one 16KB tensor, manually laid out ---
        # D ping: chunks {512,512,512,256} at 0/512/1024/1536  [banks 0-3]
        # D pong: 2048 + same                                  [banks 4-7]
        # q(t) broadcast ALSO uses the ping region: PE fills it between
        # v_mult(2t-2) (ping's last reader) and phases(2t) (which waits for
        # ACT's ln chunks to drain it). No dedicated q slots needed.
        # FT staging: pong region [2048:2304) before phases(1)
        # finale loss row: [3840:3842) (bank-7 slack, never used by D/q)
        PS = ps("PS", [128, 4096])
        Doff = lambda ph: (ph % 2) * 2048
        FTOFF = 2048
        FOFF = 3840

        # --- semaphores ---
        s_inF = sem("s_inF"); s_inW = sem("s_inW"); s_cst = sem("s_cst")
        s_wtb = [sem(f"s_wtb{i}") for i in range(3)]
        s_fn = sem("s_fn"); s_wln = sem("s_wln")
        s_sq = sem("s_sq")          # DVE squares done, per tile
        s_qc = sem("s_qc")          # PE q-chunk done, global count
        s_lnc = sem("s_lnc")        # ACT ln chunk done, global count
        s_einv = sem("s_einv")      # ACT exp-inv done, per tile
        s_D = sem("s_D"); s_mult = sem("s_mult"); s_exp = sem("s_exp")
        s_ftp = sem("s_ftp"); s_ftc = sem("s_ftc")
        s_cy = sem("s_cy"); s_ddf = sem("s_ddf"); s_db = sem("s_db")
        s_cosm = sem("s_cosm"); s_eyem = sem("s_eyem")
        s_sb = sem("s_sb"); s_tt = sem("s_tt"); s_lns = sem("s_lns")
        s_li = sem("s_li"); s_fin = sem("s_fin"); s_loss = sem("s_loss")
        s_prep = sem("s_prep"); s_rls = sem("s_rls"); s_rs = sem("s_rs")
        s_cc = sem("s_cc")
        s_gd = sem("s_gd")
        s_vh = sem("s_vh"); s_ah = sem("s_ah")

        _hs = {"v": 0, "a": 0}

        def vbar(eng, ins):
            key = "v" if eng.engine == mybir.EngineType.DVE else "a"
            s = s_vh if key == "v" else s_ah
            _hs[key] += 1
            ins.then_inc(s, 1)
            eng.wait_ge(s, _hs[key])

        with nc.Block() as block:

            # ---------------- SYNC: all input DMAs ----------------
            # consts+F first (they gate ACT norms/PE transposes), then WT0,
            # then WL (cosy path, needed ~15us later), then the WT stream.
            @block.sync
            def _(sync):
                sync.dma_start(cpack[:, :], cpack_ext.ap()).then_inc(s_cst, 16)
                sync.dma_start(cbf[:, :], cbf_ext.ap()).then_inc(s_cst, 16)
                sync.dma_start(
                    F_[:, :, :],
                    bass.AP(feat_ext, 0, [[E, 128], [128 * E, 2], [1, E]]),
                ).then_inc(s_inF, 16)
                for t in range(NTILES):
                    if t >= 3:
                        sync.wait_ge(s_sq, t - 2)        # squares of t-3 done
                        sync.wait_ge(s_D, 2 * (t - 2))   # PE phases of t-3 done
                    sync.dma_start(
                        WT[t % 3][:, :, :],
                        bass.AP(wt_ext, t * WT_TILE_ELEMS,
                                [[TILE_C, 128], [128 * TILE_C, ECH], [1, TILE_C]]),
                    ).then_inc(s_wtb[t % 3], 16)
                    if t == 0:
                        sync.dma_start(
                            WL[:, :, :],
                            bass.AP(wlab_ext, 0, [[E, 128], [128 * E, 2], [1, E]]),
                        ).then_inc(s_inW, 16)

            # ---------------- ACT (scalar) ----------------
            @block.scalar
            def _(a):
                Act = mybir.ActivationFunctionType
                neg32 = cpack[:, 129:130]
                ln32c = cpack[:, 130:131]
                # dummy exp on garbage: triggers the one ACT_TABLE_LOAD
                # (natural_log_exp set) while the input DMAs are in flight
                a.activation(SCR[:, 0:1], qfw[:, 0:1], Act.Exp, scale=0.0)
                # norms of f and wlab (squares -> ln/exp rsqrt, one act table)
                a.wait_ge(s_inF, 16)
                a.activation(SCR[:, :], F_[:, 0, :], Act.Square,
                             accum_out=qfw[:, 0:1])
                a.activation(SCR[:, :], F_[:, 1, :], Act.Square,
                             accum_out=qfw[:, 1:2])
                a.wait_ge(s_inW, 16)
                a.activation(SCR[:, :], WL[:, 0, :], Act.Square,
                             accum_out=qfw[:, 2:3])
                ins = a.activation(SCR[:, :], WL[:, 1, :], Act.Square,
                                   accum_out=qfw[:, 3:4])
                vbar(a, ins)
                ins = a.activation(lnq4[:, :], qfw[:, :], Act.Ln)
                vbar(a, ins)
                ins = a.activation(invfw[:, :], lnq4[:, :], Act.Exp, scale=-0.5)
                vbar(a, ins)
                a.activation(FN[:, 0, :], F_[:, 0, :], Act.Copy,
                             scale=invfw[:, 0:1])
                a.activation(FN[:, 1, :], F_[:, 1, :], Act.Copy,
                             scale=invfw[:, 1:2]).then_inc(s_fn, 1)
                a.activation(WLN[:, 0, :], WL[:, 0, :], Act.Copy,
                             scale=invfw[:, 2:3])
                a.activation(WLN[:, 1, :], WL[:, 1, :], Act.Copy,
                             scale=invfw[:, 3:4]).then_inc(s_wln, 1)

                def a_lninv(t):
                    # per-tile inv-norm: ln on q chunks (in the ping D region),
                    # then exp(-.5 ln + ln32)
                    for c, (co, cw) in enumerate(CHUNKS):
                        a.wait_ge(s_qc, len(CHUNKS) * t + c + 1)
                        if c == 0 and t >= 2:
                            a.wait_ge(s_einv, t - 1)   # LNB buffer free
                        ins = a.activation(
                            LNB[t % 2][:, co:co + cw],
                            PS[:, co:co + cw],
                            Act.Ln)
                        ins.then_inc(s_lnc, 1)
                    # self-wait on s_lnc = visibility barrier for LNB writes
                    a.wait_ge(s_lnc, len(CHUNKS) * (t + 1))
                    if t >= 2:
                        a.wait_ge(s_mult, 2 * t - 2)   # INVB buffer free
                    a.activation(INVB[t % 2][:, :], LNB[t % 2][:, :], Act.Exp,
                                 bias=ln32c, scale=-0.5).then_inc(s_einv, 1)

                def a_exp(ph):
                    t, half = ph // 2, ph % 2
                    a.wait_ge(s_mult, ph + 1)
                    if ph >= 2:
                        a.wait_ge(s_exp, ph - 1)  # EJ buffer visible-order
                    a.activation(
                        EJ[ph % 2][:, :], CN[ph % 2][:, :], Act.Exp,
                        bias=neg32,
                        accum_out=bass.AP(
                            SACC, half * NTILES + t,
                            [[2 * NTILES, 128], [1, 1]])).then_inc(s_exp, 1)

                a.wait_ge(s_cst, 32)    # ln32c/neg32 resident before first use
                for t in range(NTILES):
                    a_lninv(t)          # lns first: phases(2t) waits on them
                    if t >= 1:          # exps lag one tile, off PE's path
                        a_exp(2 * (t - 1))
                        a_exp(2 * (t - 1) + 1)
                a_exp(2 * (NTILES - 1))
                a_exp(2 * (NTILES - 1) + 1)

                # margin exps: only needed by the finale's adj term
                a.wait_ge(s_cosm, 1)
                a.activation(ey[:, :], cosy[:, :], Act.Exp,
                             bias=neg32, scale=SCALE)
                a.activation(em[:, :], cosm[:, :], Act.Exp,
                             bias=neg32, scale=SCALE).then_inc(s_eyem, 1)

                # finale
                a.wait_ge(s_tt, 1)
                a.activation(lS[:, :], TT[:, :], Act.Ln).then_inc(s_lns, 1)

            # ---------------- GPSIMD: bcasts, remote exchange, out ----------
            @block.gpsimd
            def _(g):
                g.load_library(library_config.proxy)
                g.collective_compute(
                    "AllGather", mybir.AluOpType.bypass,
                    replica_groups=[list(range(NCORES))],
                    ins=[cc_in.ap().opt()],
                    outs=[cc_out.ap().opt()],
                ).then_inc(s_cc, 1)
                # difficulty row broadcast for the rank/quantile compares
                g.wait_ge(s_cy, 1)
                g.dma_start(
                    bass.AP(DDR, 0, [[B, 1], [2, 128], [1, 2]]),
                    bass.AP(dd, 0, [[2, 128], [1, 2]]),
                ).then_inc(s_ddf, 16)
                g.wait_ge(s_ddf, 16)
                g.partition_broadcast(DB[:, :], DDR[0:1, :]).then_inc(s_db, 1)
                # remote S-exchange descriptors (addresses only; fired at end).
                for j in range(1, NCORES):
                    g.remote_dma_broadcast(
                        AG[:, 2 * j:2 * j + 2], Sb[:, :],
                        remote_sem=s_rs, local_sem=s_rls,
                        rdests=[(0, j) if k == j else None for k in range(NCORES)],
                        queue_num=1,
                    ).then_inc(s_prep, 1)

                # fire the S exchange once local shard sums are ready
                g.wait_ge(s_cc, 1)
                g.wait_ge(s_prep, NCORES - 1)
                g.wait_ge(s_sb, 1)
                g.trigger_dma(count=NCORES - 1, queue_num=1)
                g.wait_ge(s_loss, 1)
                g.dma_start(out_ext[:, :], loss[:, :]).then_inc(s_gd, 16)
                g.wait_ge(s_gd, 16)

            # ---------------- DVE (vector) ----------------
            @block.vector
            def _(v):
                Alu = mybir.AluOpType

                def v_sq(t):
                    v.wait_ge(s_wtb[t % 3], 16 * (t // 3 + 1))
                    if t >= 2:
                        v.wait_ge(s_qc, len(CHUNKS) * (t - 1))  # q(t-2) done
                    wtf = bass.AP(WT[t % 3], 0, [[ECH * TILE_C, 128],
                                                 [1, ECH * TILE_C]])
                    w2f = bass.AP(W2[t % 2], 0, [[ECH * TILE_C, 128],
                                                 [1, ECH * TILE_C]])
                    v.tensor_mul(w2f, wtf, wtf).then_inc(s_sq, 1)

                def v_mult(ph):
                    t = ph // 2
                    v.wait_ge(s_D, ph + 1)
                    v.wait_ge(s_einv, t + 1)
                    if ph >= 2:
                        v.wait_ge(s_exp, ph - 1)   # CN buffer free
                    v.tensor_mul(
                        CN[ph % 2][:, :],
                        PS[:, Doff(ph):Doff(ph) + TILE_C],
                        INVB[t % 2][:, :]).then_inc(s_mult, 1)

                # tile-0 squares first: nothing upstream of them but the DMA
                v_sq(0)
                # fT chunk copies (ping-pong with PE transposes via PS staging)
                for ec in range(ECH):
                    v.wait_ge(s_ftp, ec + 1)
                    v.tensor_copy(
                        bass.AP(FT, ec * 256, [[ECH * 256, 128], [1, 256]]),
                        PS[:, FTOFF:FTOFF + 256]).then_inc(s_ftc, 1)
                # cos_y (exact fp32) and difficulty
                v.wait_ge(s_wln, 1)
                for b in range(2):
                    ins = v.tensor_mul(SCR[:, :], FN[:, b, :], WLN[:, b, :])
                    vbar(v, ins)
                    ins = v.tensor_reduce(cosy[:, b:b + 1], SCR[:, :],
                                          axis=mybir.AxisListType.X, op=Alu.add)
                    vbar(v, ins)
                v.tensor_scalar(dd[:, :], cosy[:, :], -1.0, 1.0,
                                Alu.mult, Alu.add).then_inc(s_cy, 1)

                # big loop next; the margin chain below waits on the gpsimd
                # difficulty broadcast, which sits behind the launch-sync
                # collective -- putting it before the tile loop would stall
                # the whole pipeline on the slowest core's launch. v_mults
                # come BEFORE the next tile's squares: q(t+1) waits on them.
                for t in range(1, NTILES):
                    v_mult(2 * (t - 1))
                    v_mult(2 * (t - 1) + 1)
                    v_sq(t)
                v_mult(2 * (NTILES - 1))
                v_mult(2 * (NTILES - 1) + 1)

                # rank/quantile: cnt_i = #{j: d_j <= d_i}; mask = cnt >= 52
                v.wait_ge(s_db, 1)
                for b in range(2):
                    ins = v.tensor_scalar(
                        CMP2[:, :], DB[:, :], dd[:, b:b + 1], 0.0,
                        Alu.is_le, Alu.add, accum_out=cnt[:, b:b + 1])
                    vbar(v, ins)
                v.tensor_scalar(mask[:, :], cnt[:, :], 51.5, None, Alu.is_ge)
                ins = v.tensor_scalar(t1[:, :], dd[:, :], ALPHA, M_BASE,
                                      Alu.mult, Alu.add)
                vbar(v, ins)
                ins = v.tensor_scalar(m015[:, :], mask[:, :], BETA, None, Alu.mult)
                vbar(v, ins)
                ins = v.tensor_add(mm_[:, :], t1[:, :], m015[:, :])
                vbar(v, ins)
                # cmpv = -cos(pi*m) = sin(x), x = pi*m - pi/2 in [0, 1.1]:
                # odd Taylor to x^7, |err| < 6e-9 on this range
                ins = v.tensor_scalar(xs[:, :], mm_[:, :], math.pi,
                                      -math.pi / 2.0, Alu.mult, Alu.add)
                vbar(v, ins)
                ins = v.tensor_mul(x2[:, :], xs[:, :], xs[:, :])
                vbar(v, ins)
                ins = v.tensor_scalar(pa[:, :], x2[:, :], -1.0 / 5040.0,
                                      1.0 / 120.0, Alu.mult, Alu.add)
                vbar(v, ins)
                ins = v.tensor_mul(pb[:, :], pa[:, :], x2[:, :])
                vbar(v, ins)
                ins = v.tensor_scalar(pc[:, :], pb[:, :], -1.0 / 6.0, None,
                                      Alu.add)
                vbar(v, ins)
                ins = v.tensor_mul(pa[:, :], pc[:, :], x2[:, :])
                vbar(v, ins)
                ins = v.tensor_scalar(pb[:, :], pa[:, :], 1.0, None, Alu.add)
                vbar(v, ins)
                ins = v.tensor_mul(cmpv[:, :], pb[:, :], xs[:, :])
                vbar(v, ins)
                v.tensor_mul(t2[:, :], cosy[:, :], cmpv[:, :])
                ins = v.tensor_scalar(t3[:, :], mm_[:, :], -SIN_M, None, Alu.mult)
                vbar(v, ins)
                v.tensor_sub(cosm[:, :], t3[:, :], t2[:, :]).then_inc(s_cosm, 1)
                v.wait_ge(s_cosm, 1)  # self-wait doubles as visibility barrier
                v.tensor_scalar(li0c[:, :], cosm[:, :], -SCALE, SCALE,
                                Alu.mult, Alu.add)

                # finale: local shard sum, remote exchange, loss
                v.wait_ge(s_eyem, 1)
                v.tensor_sub(adj[:, :], em[:, :], ey[:, :])
                v.wait_ge(s_exp, NPH)
                ins = v.tensor_reduce(
                    Sb[:, :],
                    bass.AP(SACC, 0, [[2 * NTILES, 128], [NTILES, 2], [1, NTILES]]),
                    axis=mybir.AxisListType.X, op=Alu.add)
                ins.then_inc(s_sb, 1)
                v.wait_ge(s_rs, 2 * (NCORES - 1))   # all 7 peers' slots landed
                ins = v.tensor_reduce(
                    Sf1[:, :],
                    bass.AP(AG, 2, [[2 * NCORES, 128], [1, 2], [2, NCORES - 1]]),
                    axis=mybir.AxisListType.X, op=Alu.add)
                vbar(v, ins)
                ins = v.tensor_add(Sfull[:, :], Sf1[:, :], Sb[:, :])
                vbar(v, ins)
                v.tensor_add(TT[:, :], Sfull[:, :], adj[:, :]).then_inc(s_tt, 1)
                v.wait_ge(s_lns, 1)
                v.tensor_add(li[:, :], lS[:, :], li0c[:, :]).then_inc(s_li, 1)
                v.wait_ge(s_fin, 1)
                ins = v.tensor_reduce(lsum[:, :], PS[0:1, FOFF:FOFF + 2],
                                      axis=mybir.AxisListType.X, op=Alu.add)
                vbar(v, ins)
                v.tensor_scalar(loss[:, :], lsum[:, :], 1.0 / B, None,
                                Alu.mult).then_inc(s_loss, 1)

            # ---------------- PE (tensor) ----------------
            @block.tensor
            def _(te):
                te.wait_ge(s_cst, 32)
                te.wait_ge(s_fn, 1)
                identf = cpack[:, 0:128]   # fp32 identity for the transposes
                onesf = cpack[:, 128:129]  # fp32 ones column (finale mean)
                onesq = cbf[:, 0:128]      # bf16 all-ones 128x128 (norm bcast)
                # fT = transpose(f_norm): [e_p, ec, bc, b] via PS staging
                for ec in range(ECH):
                    if ec >= 1:
                        te.wait_ge(s_ftc, ec)
                    te.transpose(PS[:, FTOFF:FTOFF + 128],
                                 FN[:, 0, ec * 128:(ec + 1) * 128], identf)
                    te.transpose(PS[:, FTOFF + 128:FTOFF + 256],
                                 FN[:, 1, ec * 128:(ec + 1) * 128],
                                 identf).then_inc(s_ftp, 1)

                def t_qmm(t):
                    # norm^2 broadcast: ones^T @ W2 chunks, accumulated over
                    # ec, written into the ping D region (free after
                    # v_mult(2t-2); phases(2t) waits for ACT's lns to drain it)
                    te.wait_ge(s_sq, t + 1)
                    if t >= 1:
                        te.wait_ge(s_mult, 2 * t - 1)   # ping region free
                    for c, (co, cw) in enumerate(CHUNKS):
                        for ec in range(ECH):
                            ins = te.matmul(
                                PS[:, co:co + cw],
                                onesq,
                                W2[t % 2][:, ec, co:co + cw],
                                start=(ec == 0), stop=(ec == ECH - 1),
                                skip_group_check=True)
                        ins.then_inc(s_qc, 1)

                def t_phase(ph):
                    t, half = ph // 2, ph % 2
                    if half == 0:
                        te.wait_ge(s_wtb[t % 3], 16 * (t // 3 + 1))
                        # ping region: ACT lns of q(t) must have drained it
                        te.wait_ge(s_lnc, len(CHUNKS) * (t + 1))
                    else:
                        if ph >= 2:
                            te.wait_ge(s_mult, ph - 1)  # pong D free
                    if ph == 1:
                        te.wait_ge(s_ftc, ECH)          # FT staging drained
                    for (co, cw) in CHUNKS:
                        for ec in range(ECH):
                            ins = te.matmul(
                                PS[:, Doff(ph) + co:Doff(ph) + co + cw],
                                FT[:, ec, half, :],
                                WT[t % 3][:, ec, co:co + cw],
                                start=(ec == 0), stop=(ec == ECH - 1),
                                skip_group_check=True)
                    ins.then_inc(s_D, 1)

                for t in range(NTILES):
                    t_qmm(t)
                    t_phase(2 * t)
                    t_phase(2 * t + 1)

                # finale: batch mean via ones-stationary matmul into q slack
                te.wait_ge(s_li, 1)
                te.matmul(PS[0:1, FOFF:FOFF + 2], onesf, li[:, :]).then_inc(s_fin, 1)

        return nc


def _shard_host(features, labels, kernel_w):
    """Host-side shard + pack (layout + bf16 cast only, no arithmetic)."""
    import ml_dtypes
    features = np.ascontiguousarray(features, dtype=np.float32)
    kernel_w = np.ascontiguousarray(kernel_w, dtype=np.float32)
    labels = np.asarray(labels).astype(np.int64)
    wlab = np.ascontiguousarray(kernel_w[labels])        # (B, E) gather
    pad_row = -features[0]                               # direction only matters
    cpack = np.zeros((128, 132), dtype=np.float32)
    cpack[:, 0:128] = np.eye(128, dtype=np.float32)
    cpack[:, 128] = 1.0          # onesf
    cpack[:, 129] = -SCALE       # neg32
    cpack[:, 130] = LN32         # ln32c
    cbf = np.ones((128, 128), dtype=np.float32).astype(ml_dtypes.bfloat16)
    in_maps = []
    cpc_raw = C // NCORES                                # 12500
    for c in range(NCORES):
        shard = kernel_w[c * cpc_raw:(c + 1) * cpc_raw]  # (12500, E)
        pad = np.broadcast_to(pad_row, (CPC - cpc_raw, E))
        shard = np.concatenate([shard, pad], axis=0)     # (12544, E)
        # (CPC, E) -> transpose -> (E, CPC) -> (ECH,128, NTILES,TILE_C)
        wt = shard.T.reshape(ECH, 128, NTILES, TILE_C)
        wt = np.ascontiguousarray(wt.transpose(2, 0, 1, 3)).astype(ml_dtypes.bfloat16)
        in_maps.append({"wt": wt, "feat": features, "wlab": wlab,
                        "cpack": cpack, "cbf": cbf})
    return in_maps


def _get_nc():
    if "nc" not in _CACHE:
        nc = build_nc()
        from concourse.library_overlay import lower_extended_insts
        lower_extended_insts(nc)
        _CACHE["nc"] = nc
    return _CACHE["nc"]


def kernel(features, labels, kernel):
    in_maps = _shard_host(features, labels, kernel)
    nc = _get_nc()
    res = run_bass_kernel_spmd(nc, in_maps, core_ids=list(range(NCORES)))
    out = res.results[0]["out"]
    return np.float32(out.reshape(())[()])


# revision 24
# speedup vs baseline: 1.2355x; 1.2355x over previous
"""AdaArcFace loss on 8 TRN2 NeuronCores (Bass, class-sharded tensor parallel).

loss = mean_i( LSE_i - 32*cosm_i ),  LSE_i = 32 + ln(S_i + em_i - ey_i)
  S_i  = sum_c exp(32*(cos[i,c] - 1))   <- the only term needing the big matmul
  cos_y/quantile/margin path is tiny, exact fp32, replicated on every core.

Sharding: 100000 classes -> 8 cores x 12544 (44 pad rows = -features[0], whose
softmax contribution is ~1e-17 relative). kernel shard is cast to bf16 and
transposed on host (pack only) so DMA streams contiguous at half the bytes and
the PE gets emb-on-partitions.

v4 (from v3's 223-278us baseline; trace showed PE half-clocked by HAM, ACT
square pass + SWDGE flatten dominating):
 - weights stream as bf16: DMA 72->37us, FWL on the bf16 stationary.
 - norm^2 via ones-stationary broadcast matmul accumulating the 4 emb chunks
   of W2 straight into PSUM q-slots: kills the ACT square pass, DVE pairsums,
   the 98 one-row qmm matmuls, and the 7392-packet SWDGE flatten/broadcast.
 - inv = exp(-.5 ln q + ln32) on [128,256] q-chunks; everything ACT ever runs
   (Square/Ln/Exp/Copy) lives in the one natural_log_exp table set. cos(m*pi)
   moved to a DVE sin-polynomial so the trig table never loads.
 - PE is the only saturated engine -> HAM stays at 8/8 after warmup.
 - squares on DVE bf16 (2x mode), v_mult per phase off PSUM, exp+accum on ACT.
"""

import math
import numpy as np

import concourse.bass as bass
import concourse.mybir as mybir
from concourse import library_config
from concourse.bass_utils import run_bass_kernel_spmd

F32 = mybir.dt.float32
BF16 = mybir.dt.bfloat16

# problem constants (hardcoded per harness contract)
B = 256          # batch
E = 512          # embedding
C = 100000       # classes
NCORES = 8
CPC = 12544      # classes per core (padded): 98 groups of 128
TILE_C = 1792    # classes per stream tile
NTILES = CPC // TILE_C             # 7
NPH = 2 * NTILES                   # 14 phases (bc-halves)
QW = 256                           # q broadcast chunk width
NQC = TILE_C // QW                 # 7 q-chunks per tile
CHUNKS = [(0, 512), (512, 512), (1024, 512), (1536, 256)]  # D chunk (off, w)
SCALE = 32.0
M_BASE = 0.5
ALPHA = 0.1
BETA = 0.15
SIN_M = math.sin(M_BASE)
LN32 = math.log(32.0)
ECH = E // 128   # 4 emb chunks

_CACHE = {}


def build_nc():
    nc = bass.Bass(target_bir_lowering=False, num_devices=NCORES, num_swdge_queues=2)

    wt_ext = nc.declare_dram_parameter(
        "wt", [NTILES, ECH, 128, TILE_C], BF16, isOutput=False)
    feat_ext = nc.declare_dram_parameter("feat", [B, E], F32, isOutput=False)
    wlab_ext = nc.declare_dram_parameter("wlab", [B, E], F32, isOutput=False)
    cpack_ext = nc.declare_dram_parameter("cpack", [128, 132], F32, isOutput=False)
    cbf_ext = nc.declare_dram_parameter("cbf", [128, 128], BF16, isOutput=False)
    out_ext = nc.declare_dram_parameter("out", [1, 1], F32, isOutput=True)

    # dummy collective: forces the runtime's synchronized multi-core launch
    cc_in = nc.dram_tensor("cc_in", [1, 1], F32)
    cc_out = nc.dram_tensor("cc_out", [1, NCORES], F32, addr_space="Shared")

    WT_TILE_ELEMS = ECH * 128 * TILE_C

    from contextlib import ExitStack
    ctx = ExitStack()
    sb = lambda name, shape, dt=F32: ctx.enter_context(nc.sbuf_tensor(name, shape, dt))
    ps = lambda name, shape, dt=F32: ctx.enter_context(nc.psum_tensor(name, shape, dt))
    sem = lambda name: ctx.enter_context(nc.semaphore(name))

    with ctx:
        # --- SBUF ---
        WT = [sb(f"WT{i}", [128, ECH, TILE_C], BF16) for i in range(3)]
        W2 = [sb(f"W2{i}", [128, ECH, TILE_C], BF16) for i in range(2)]
        LNB = [sb(f"LNB{i}", [128, TILE_C]) for i in range(2)]
        INVB = [sb(f"INVB{i}", [128, TILE_C]) for i in range(2)]  # 32/||w||
        CN = [sb(f"CN{i}", [128, TILE_C], BF16) for i in range(2)]  # 32*cos
        EJ = [sb(f"EJ{i}", [128, TILE_C], BF16) for i in range(2)]  # exp junk
        SACC = sb("SACC", [128, 2, NTILES])                     # accum_out slots
        FT = sb("FT", [128, ECH, 2, 128], BF16)                 # fhatT bf16
        F_ = sb("F", [128, 2, E])
        FN = sb("FN", [128, 2, E])
        WL = sb("WL", [128, 2, E])
        WLN = sb("WLN", [128, 2, E])
        SCR = sb("SCR", [128, E])      # square/cosy scratch
        CMP2 = sb("CMP2", [128, B])
        DB = sb("DB", [128, B])        # difficulty broadcast (rank compares)
        DDR = sb("DDR", [1, B])        # difficulty flattened row
        AG = sb("AG", [128, 2 * NCORES])  # remote slot j at cols [2j, 2j+2)
        qfw = sb("qfw", [128, 4])
        lnq4 = sb("lnq4", [128, 4])
        invfw = sb("invfw", [128, 4])
        cosy = sb("cosy", [128, 2]); dd = sb("dd", [128, 2])
        cnt = sb("cnt", [128, 2]); mask = sb("mask", [128, 2])
        t1 = sb("t1", [128, 2]); m015 = sb("m015", [128, 2]); mm_ = sb("mm", [128, 2])
        xs = sb("xs", [128, 2]); x2 = sb("x2", [128, 2])
        pa = sb("pa", [128, 2]); pb = sb("pb", [128, 2]); pc = sb("pc", [128, 2])
        cmpv = sb("cmpv", [128, 2]); t2 = sb("t2", [128, 2]); t3 = sb("t3", [128, 2])
        cosm = sb("cosm", [128, 2]); ey = sb("ey", [128, 2]); em = sb("em", [128, 2])
        adj = sb("adj", [128, 2]); li0c = sb("li0c", [128, 2])
        Sb = sb("Sb", [128, 2]); Sf1 = sb("Sf1", [128, 2])
        Sfull = sb("Sfull", [128, 2]); TT = sb("TT", [128, 2])
        lS = sb("lS", [128, 2]); li = sb("li", [128, 2])
        lsum = sb("lsum", [1, 1]); loss = sb("loss", [1, 1])
        cpack = sb("cpk", [128, 132])
        cbf = sb("cbfs", [128, 128], BF16)

        # --- PSUM: one 16KB tensor, manually laid out ---
        # D ping: chunks {512,512,512,256} at 0/512/1024/1536  [banks 0-3]
        # D pong: 2048 + same                                  [banks 4-7]
        # q(t) broadcast ALSO uses the ping region: PE fills it between
        # v_mult(2t-2) (ping's last reader) and phases(2t) (which waits for
        # ACT's ln chunks to drain it). No dedicated q slots needed.
        # FT staging: pong region [2048:2304) before phases(1)
        # finale loss row: [3840:3842) (bank-7 slack, never used by D/q)
        PS = ps("PS", [128, 4096])
        Doff = lambda ph: (ph % 2) * 2048
        FTOFF = 2048
        FOFF = 3840

        # --- semaphores ---
        s_inF = sem("s_inF"); s_inW = sem("s_inW"); s_cst = sem("s_cst")
        s_wtb = [sem(f"s_wtb{i}") for i in range(3)]
        s_fn = sem("s_fn"); s_wln = sem("s_wln")
        s_sq = sem("s_sq")          # DVE squares done, per tile
        s_qc = sem("s_qc")          # PE q-chunk done, global count
        s_lnc = sem("s_lnc")        # ACT ln chunk done, global count
        s_einv = sem("s_einv")      # ACT exp-inv done, per tile
        s_D = sem("s_D"); s_mult = sem("s_mult"); s_exp = sem("s_exp")
        s_ftp = sem("s_ftp"); s_ftc = sem("s_ftc")
        s_cy = sem("s_cy"); s_ddf = sem("s_ddf"); s_db = sem("s_db")
        s_cosm = sem("s_cosm"); s_eyem = sem("s_eyem")
        s_sb = sem("s_sb"); s_tt = sem("s_tt"); s_lns = sem("s_lns")
        s_li = sem("s_li"); s_fin = sem("s_fin"); s_loss = sem("s_loss")
        s_prep = sem("s_prep"); s_rls = sem("s_rls"); s_rs = sem("s_rs")
        s_cc = sem("s_cc")
        s_gd = sem("s_gd")
        s_vh = sem("s_vh"); s_ah = sem("s_ah")

        _hs = {"v": 0, "a": 0}

        def vbar(eng, ins):
            key = "v" if eng.engine == mybir.EngineType.DVE else "a"
            s = s_vh if key == "v" else s_ah
            _hs[key] += 1
            ins.then_inc(s, 1)
            eng.wait_ge(s, _hs[key])

        with nc.Block() as block:

            # ---------------- SYNC: all input DMAs ----------------
            # consts+F first (they gate ACT norms/PE transposes), then WT0,
            # then WL (cosy path, needed ~15us later), then the WT stream.
            @block.sync
            def _(sync):
                sync.dma_start(cpack[:, :], cpack_ext.ap()).then_inc(s_cst, 16)
                sync.dma_start(cbf[:, :], cbf_ext.ap()).then_inc(s_cst, 16)
                for t in range(NTILES):
                    if t >= 3:
                        sync.wait_ge(s_sq, t - 2)        # squares of t-3 done
                        sync.wait_ge(s_D, 2 * (t - 2))   # PE phases of t-3 done
                    sync.dma_start(
                        WT[t % 3][:, :, :],
                        bass.AP(wt_ext, t * WT_TILE_ELEMS,
                                [[TILE_C, 128], [128 * TILE_C, ECH], [1, TILE_C]]),
                    ).then_inc(s_wtb[t % 3], 16)
                    if t == 0:
                        # F right behind WT0 (norms/transposes), WL after
                        sync.dma_start(
                            F_[:, :, :],
                            bass.AP(feat_ext, 0,
                                    [[E, 128], [128 * E, 2], [1, E]]),
                        ).then_inc(s_inF, 16)
                        sync.dma_start(
                            WL[:, :, :],
                            bass.AP(wlab_ext, 0,
                                    [[E, 128], [128 * E, 2], [1, E]]),
                        ).then_inc(s_inW, 16)

            # ---------------- ACT (scalar) ----------------
            @block.scalar
            def _(a):
                Act = mybir.ActivationFunctionType
                neg32 = cpack[:, 129:130]
                ln32c = cpack[:, 130:131]
                # dummy exp on garbage: triggers the one ACT_TABLE_LOAD
                # (natural_log_exp set) while the input DMAs are in flight
                a.activation(SCR[:, 0:1], qfw[:, 0:1], Act.Exp, scale=0.0)
                # f norms first (gates the PE transposes); wlab norms separate
                # so FN doesn't wait on the later WL DMA
                a.wait_ge(s_inF, 16)
                a.activation(SCR[:, :], F_[:, 0, :], Act.Square,
                             accum_out=qfw[:, 0:1])
                ins = a.activation(SCR[:, :], F_[:, 1, :], Act.Square,
                                   accum_out=qfw[:, 1:2])
                vbar(a, ins)
                ins = a.activation(lnq4[:, 0:2], qfw[:, 0:2], Act.Ln)
                vbar(a, ins)
                ins = a.activation(invfw[:, 0:2], lnq4[:, 0:2], Act.Exp,
                                   scale=-0.5)
                vbar(a, ins)
                a.activation(FN[:, 0, :], F_[:, 0, :], Act.Copy,
                             scale=invfw[:, 0:1])
                a.activation(FN[:, 1, :], F_[:, 1, :], Act.Copy,
                             scale=invfw[:, 1:2]).then_inc(s_fn, 1)
                a.wait_ge(s_inW, 16)
                a.activation(SCR[:, :], WL[:, 0, :], Act.Square,
                             accum_out=qfw[:, 2:3])
                ins = a.activation(SCR[:, :], WL[:, 1, :], Act.Square,
                                   accum_out=qfw[:, 3:4])
                vbar(a, ins)
                ins = a.activation(lnq4[:, 2:4], qfw[:, 2:4], Act.Ln)
                vbar(a, ins)
                ins = a.activation(invfw[:, 2:4], lnq4[:, 2:4], Act.Exp,
                                   scale=-0.5)
                vbar(a, ins)
                a.activation(WLN[:, 0, :], WL[:, 0, :], Act.Copy,
                             scale=invfw[:, 2:3])
                a.activation(WLN[:, 1, :], WL[:, 1, :], Act.Copy,
                             scale=invfw[:, 3:4]).then_inc(s_wln, 1)

                def a_lninv(t):
                    # per-tile inv-norm: ln on q chunks (in the ping D region),
                    # then exp(-.5 ln + ln32)
                    for c, (co, cw) in enumerate(CHUNKS):
                        a.wait_ge(s_qc, len(CHUNKS) * t + c + 1)
                        if c == 0 and t >= 2:
                            a.wait_ge(s_einv, t - 1)   # LNB buffer free
                        ins = a.activation(
                            LNB[t % 2][:, co:co + cw],
                            PS[:, co:co + cw],
                            Act.Ln)
                        ins.then_inc(s_lnc, 1)
                    # self-wait on s_lnc = visibility barrier for LNB writes
                    a.wait_ge(s_lnc, len(CHUNKS) * (t + 1))
                    if t >= 2:
                        a.wait_ge(s_mult, 2 * t - 2)   # INVB buffer free
                    a.activation(INVB[t % 2][:, :], LNB[t % 2][:, :], Act.Exp,
                                 bias=ln32c, scale=-0.5).then_inc(s_einv, 1)

                def a_exp(ph):
                    t, half = ph // 2, ph % 2
                    a.wait_ge(s_mult, ph + 1)
                    if ph >= 2:
                        a.wait_ge(s_exp, ph - 1)  # EJ buffer visible-order
                    a.activation(
                        EJ[ph % 2][:, :], CN[ph % 2][:, :], Act.Exp,
                        bias=neg32,
                        accum_out=bass.AP(
                            SACC, half * NTILES + t,
                            [[2 * NTILES, 128], [1, 1]])).then_inc(s_exp, 1)

                a.wait_ge(s_cst, 32)    # ln32c/neg32 resident before first use
                for t in range(NTILES):
                    a_lninv(t)          # lns first: phases(2t) waits on them
                    if t >= 1:          # exps lag one tile, off PE's path
                        a_exp(2 * (t - 1))
                        a_exp(2 * (t - 1) + 1)
                a_exp(2 * (NTILES - 1))
                a_exp(2 * (NTILES - 1) + 1)

                # margin exps: only needed by the finale's adj term
                a.wait_ge(s_cosm, 1)
                a.activation(ey[:, :], cosy[:, :], Act.Exp,
                             bias=neg32, scale=SCALE)
                a.activation(em[:, :], cosm[:, :], Act.Exp,
                             bias=neg32, scale=SCALE).then_inc(s_eyem, 1)

                # finale
                a.wait_ge(s_tt, 1)
                a.activation(lS[:, :], TT[:, :], Act.Ln).then_inc(s_lns, 1)

            # ---------------- GPSIMD: bcasts, remote exchange, out ----------
            @block.gpsimd
            def _(g):
                g.load_library(library_config.proxy)
                g.collective_compute(
                    "AllGather", mybir.AluOpType.bypass,
                    replica_groups=[list(range(NCORES))],
                    ins=[cc_in.ap().opt()],
                    outs=[cc_out.ap().opt()],
                ).then_inc(s_cc, 1)
                # difficulty row broadcast for the rank/quantile compares
                g.wait_ge(s_cy, 1)
                g.dma_start(
                    bass.AP(DDR, 0, [[B, 1], [2, 128], [1, 2]]),
                    bass.AP(dd, 0, [[2, 128], [1, 2]]),
                ).then_inc(s_ddf, 16)
                g.wait_ge(s_ddf, 16)
                g.partition_broadcast(DB[:, :], DDR[0:1, :]).then_inc(s_db, 1)
                # remote S-exchange descriptors (addresses only; fired at end).
                for j in range(1, NCORES):
                    g.remote_dma_broadcast(
                        AG[:, 2 * j:2 * j + 2], Sb[:, :],
                        remote_sem=s_rs, local_sem=s_rls,
                        rdests=[(0, j) if k == j else None for k in range(NCORES)],
                        queue_num=1,
                    ).then_inc(s_prep, 1)

                # fire the S exchange once local shard sums are ready
                g.wait_ge(s_cc, 1)
                g.wait_ge(s_prep, NCORES - 1)
                g.wait_ge(s_sb, 1)
                g.trigger_dma(count=NCORES - 1, queue_num=1)
                g.wait_ge(s_loss, 1)
                g.dma_start(out_ext[:, :], loss[:, :]).then_inc(s_gd, 16)
                g.wait_ge(s_gd, 16)

            # ---------------- DVE (vector) ----------------
            @block.vector
            def _(v):
                Alu = mybir.AluOpType

                def v_sq(t):
                    v.wait_ge(s_wtb[t % 3], 16 * (t // 3 + 1))
                    if t >= 2:
                        v.wait_ge(s_qc, len(CHUNKS) * (t - 1))  # q(t-2) done
                    wtf = bass.AP(WT[t % 3], 0, [[ECH * TILE_C, 128],
                                                 [1, ECH * TILE_C]])
                    w2f = bass.AP(W2[t % 2], 0, [[ECH * TILE_C, 128],
                                                 [1, ECH * TILE_C]])
                    v.tensor_mul(w2f, wtf, wtf).then_inc(s_sq, 1)

                def v_mult(ph):
                    t = ph // 2
                    v.wait_ge(s_D, ph + 1)
                    v.wait_ge(s_einv, t + 1)
                    if ph >= 2:
                        v.wait_ge(s_exp, ph - 1)   # CN buffer free
                    v.tensor_mul(
                        CN[ph % 2][:, :],
                        PS[:, Doff(ph):Doff(ph) + TILE_C],
                        INVB[t % 2][:, :]).then_inc(s_mult, 1)

                # tile-0 squares first: nothing upstream of them but the DMA
                v_sq(0)
                # fT chunk copies (ping-pong with PE transposes via PS staging)
                for ec in range(ECH):
                    v.wait_ge(s_ftp, ec + 1)
                    v.tensor_copy(
                        bass.AP(FT, ec * 256, [[ECH * 256, 128], [1, 256]]),
                        PS[:, FTOFF:FTOFF + 256]).then_inc(s_ftc, 1)
                # cos_y (exact fp32) and difficulty
                v.wait_ge(s_wln, 1)
                for b in range(2):
                    ins = v.tensor_mul(SCR[:, :], FN[:, b, :], WLN[:, b, :])
                    vbar(v, ins)
                    ins = v.tensor_reduce(cosy[:, b:b + 1], SCR[:, :],
                                          axis=mybir.AxisListType.X, op=Alu.add)
                    vbar(v, ins)
                v.tensor_scalar(dd[:, :], cosy[:, :], -1.0, 1.0,
                                Alu.mult, Alu.add).then_inc(s_cy, 1)

                # big loop next; the margin chain below waits on the gpsimd
                # difficulty broadcast, which sits behind the launch-sync
                # collective -- putting it before the tile loop would stall
                # the whole pipeline on the slowest core's launch. Squares
                # run one tile ahead, BETWEEN the previous tile's v_mults,
                # so neither q(t) (needs s_sq) nor phases-pong (needs
                # s_mult) ever waits on the DVE queue position.
                v_sq(1)
                for t in range(1, NTILES):
                    v_mult(2 * (t - 1))
                    if t + 1 < NTILES:
                        v_sq(t + 1)
                    v_mult(2 * (t - 1) + 1)
                v_mult(2 * (NTILES - 1))
                v_mult(2 * (NTILES - 1) + 1)

                # rank/quantile: cnt_i = #{j: d_j <= d_i}; mask = cnt >= 52
                v.wait_ge(s_db, 1)
                for b in range(2):
                    ins = v.tensor_scalar(
                        CMP2[:, :], DB[:, :], dd[:, b:b + 1], 0.0,
                        Alu.is_le, Alu.add, accum_out=cnt[:, b:b + 1])
                    vbar(v, ins)
                v.tensor_scalar(mask[:, :], cnt[:, :], 51.5, None, Alu.is_ge)
                ins = v.tensor_scalar(t1[:, :], dd[:, :], ALPHA, M_BASE,
                                      Alu.mult, Alu.add)
                vbar(v, ins)
                ins = v.tensor_scalar(m015[:, :], mask[:, :], BETA, None, Alu.mult)
                vbar(v, ins)
                ins = v.tensor_add(mm_[:, :], t1[:, :], m015[:, :])
                vbar(v, ins)
                # cmpv = -cos(pi*m) = sin(x), x = pi*m - pi/2 in [0, 1.1]:
                # odd Taylor to x^7, |err| < 6e-9 on this range
                ins = v.tensor_scalar(xs[:, :], mm_[:, :], math.pi,
                                      -math.pi / 2.0, Alu.mult, Alu.add)
                vbar(v, ins)
                ins = v.tensor_mul(x2[:, :], xs[:, :], xs[:, :])
                vbar(v, ins)
                ins = v.tensor_scalar(pa[:, :], x2[:, :], -1.0 / 5040.0,
                                      1.0 / 120.0, Alu.mult, Alu.add)
                vbar(v, ins)
                ins = v.tensor_mul(pb[:, :], pa[:, :], x2[:, :])
                vbar(v, ins)
                ins = v.tensor_scalar(pc[:, :], pb[:, :], -1.0 / 6.0, None,
                                      Alu.add)
                vbar(v, ins)
                ins = v.tensor_mul(pa[:, :], pc[:, :], x2[:, :])
                vbar(v, ins)
                ins = v.tensor_scalar(pb[:, :], pa[:, :], 1.0, None, Alu.add)
                vbar(v, ins)
                ins = v.tensor_mul(cmpv[:, :], pb[:, :], xs[:, :])
                vbar(v, ins)
                v.tensor_mul(t2[:, :], cosy[:, :], cmpv[:, :])
                ins = v.tensor_scalar(t3[:, :], mm_[:, :], -SIN_M, None, Alu.mult)
                vbar(v, ins)
                v.tensor_sub(cosm[:, :], t3[:, :], t2[:, :]).then_inc(s_cosm, 1)
                v.wait_ge(s_cosm, 1)  # self-wait doubles as visibility barrier
                v.tensor_scalar(li0c[:, :], cosm[:, :], -SCALE, SCALE,
                                Alu.mult, Alu.add)

                # finale: local shard sum, remote exchange, loss
                v.wait_ge(s_eyem, 1)
                v.tensor_sub(adj[:, :], em[:, :], ey[:, :])
                v.wait_ge(s_exp, NPH)
                ins = v.tensor_reduce(
                    Sb[:, :],
                    bass.AP(SACC, 0, [[2 * NTILES, 128], [NTILES, 2], [1, NTILES]]),
                    axis=mybir.AxisListType.X, op=Alu.add)
                ins.then_inc(s_sb, 1)
                v.wait_ge(s_rs, 2 * (NCORES - 1))   # all 7 peers' slots landed
                ins = v.tensor_reduce(
                    Sf1[:, :],
                    bass.AP(AG, 2, [[2 * NCORES, 128], [1, 2], [2, NCORES - 1]]),
                    axis=mybir.AxisListType.X, op=Alu.add)
                vbar(v, ins)
                ins = v.tensor_add(Sfull[:, :], Sf1[:, :], Sb[:, :])
                vbar(v, ins)
                v.tensor_add(TT[:, :], Sfull[:, :], adj[:, :]).then_inc(s_tt, 1)
                v.wait_ge(s_lns, 1)
                v.tensor_add(li[:, :], lS[:, :], li0c[:, :]).then_inc(s_li, 1)
                v.wait_ge(s_fin, 1)
                ins = v.tensor_reduce(lsum[:, :], PS[0:1, FOFF:FOFF + 2],
                                      axis=mybir.AxisListType.X, op=Alu.add)
                vbar(v, ins)
                v.tensor_scalar(loss[:, :], lsum[:, :], 1.0 / B, None,
                                Alu.mult).then_inc(s_loss, 1)

            # ---------------- PE (tensor) ----------------
            @block.tensor
            def _(te):
                te.wait_ge(s_cst, 32)
                te.wait_ge(s_fn, 1)
                identf = cpack[:, 0:128]   # fp32 identity for the transposes
                onesf = cpack[:, 128:129]  # fp32 ones column (finale mean)
                onesq = cbf[:, 0:128]      # bf16 all-ones 128x128 (norm bcast)
                # fT = transpose(f_norm): [e_p, ec, bc, b] via PS staging
                for ec in range(ECH):
                    if ec >= 1:
                        te.wait_ge(s_ftc, ec)
                    te.transpose(PS[:, FTOFF:FTOFF + 128],
                                 FN[:, 0, ec * 128:(ec + 1) * 128], identf)
                    te.transpose(PS[:, FTOFF + 128:FTOFF + 256],
                                 FN[:, 1, ec * 128:(ec + 1) * 128],
                                 identf).then_inc(s_ftp, 1)

                def t_qmm(t):
                    # norm^2 broadcast: ones^T @ W2 chunks, accumulated over
                    # ec, written into the ping D region (free after
                    # v_mult(2t-2); phases(2t) waits for ACT's lns to drain it)
                    te.wait_ge(s_sq, t + 1)
                    if t >= 1:
                        te.wait_ge(s_mult, 2 * t - 1)   # ping region free
                    for c, (co, cw) in enumerate(CHUNKS):
                        for ec in range(ECH):
                            ins = te.matmul(
                                PS[:, co:co + cw],
                                onesq,
                                W2[t % 2][:, ec, co:co + cw],
                                start=(ec == 0), stop=(ec == ECH - 1),
                                skip_group_check=True)
                        ins.then_inc(s_qc, 1)

                def t_phase(ph):
                    t, half = ph // 2, ph % 2
                    if half == 0:
                        te.wait_ge(s_wtb[t % 3], 16 * (t // 3 + 1))
                        # ping region: ACT lns of q(t) must have drained it
                        te.wait_ge(s_lnc, len(CHUNKS) * (t + 1))
                    else:
                        if ph >= 2:
                            te.wait_ge(s_mult, ph - 1)  # pong D free
                    if ph == 1:
                        te.wait_ge(s_ftc, ECH)          # FT staging drained
                    for (co, cw) in CHUNKS:
                        for ec in range(ECH):
                            ins = te.matmul(
                                PS[:, Doff(ph) + co:Doff(ph) + co + cw],
                                FT[:, ec, half, :],
                                WT[t % 3][:, ec, co:co + cw],
                                start=(ec == 0), stop=(ec == ECH - 1),
                                skip_group_check=True)
                    ins.then_inc(s_D, 1)

                for t in range(NTILES):
                    t_qmm(t)
                    t_phase(2 * t)
                    t_phase(2 * t + 1)

                # finale: batch mean via ones-stationary matmul into q slack
                te.wait_ge(s_li, 1)
                te.matmul(PS[0:1, FOFF:FOFF + 2], onesf, li[:, :]).then_inc(s_fin, 1)

        return nc


def _shard_host(features, labels, kernel_w):
    """Host-side shard + pack (layout + bf16 cast only, no arithmetic)."""
    import ml_dtypes
    features = np.ascontiguousarray(features, dtype=np.float32)
    kernel_w = np.ascontiguousarray(kernel_w, dtype=np.float32)
    labels = np.asarray(labels).astype(np.int64)
    wlab = np.ascontiguousarray(kernel_w[labels])        # (B, E) gather
    pad_row = -features[0]                               # direction only matters
    cpack = np.zeros((128, 132), dtype=np.float32)
    cpack[:, 0:128] = np.eye(128, dtype=np.float32)
    cpack[:, 128] = 1.0          # onesf
    cpack[:, 129] = -SCALE       # neg32
    cpack[:, 130] = LN32         # ln32c
    cbf = np.ones((128, 128), dtype=np.float32).astype(ml_dtypes.bfloat16)
    in_maps = []
    cpc_raw = C // NCORES                                # 12500
    for c in range(NCORES):
        shard = kernel_w[c * cpc_raw:(c + 1) * cpc_raw]  # (12500, E)
        pad = np.broadcast_to(pad_row, (CPC - cpc_raw, E))
        shard = np.concatenate([shard, pad], axis=0)     # (12544, E)
        # (CPC, E) -> transpose -> (E, CPC) -> (ECH,128, NTILES,TILE_C)
        wt = shard.T.reshape(ECH, 128, NTILES, TILE_C)
        wt = np.ascontiguousarray(wt.transpose(2, 0, 1, 3)).astype(ml_dtypes.bfloat16)
        in_maps.append({"wt": wt, "feat": features, "wlab": wlab,
                        "cpack": cpack, "cbf": cbf})
    return in_maps


def _get_nc():
    if "nc" not in _CACHE:
        nc = build_nc()
        from concourse.library_overlay import lower_extended_insts
        lower_extended_insts(nc)
        _CACHE["nc"] = nc
    return _CACHE["nc"]


def kernel(features, labels, kernel):
    in_maps = _shard_host(features, labels, kernel)
    nc = _get_nc()
    res = run_bass_kernel_spmd(nc, in_maps, core_ids=list(range(NCORES)))
    out = res.results[0]["out"]
    return np.float32(out.reshape(())[()])


# revision 29
# speedup vs baseline: 1.3062x; 1.0572x over previous
"""AdaArcFace loss on 8 TRN2 NeuronCores (Bass, class-sharded tensor parallel).

loss = mean_i( LSE_i - 32*cosm_i ),  LSE_i = 32 + ln(S_i + em_i - ey_i)
  S_i  = sum_c exp(32*(cos[i,c] - 1))   <- the only term needing the big matmul
  cos_y/quantile/margin path is tiny, exact fp32, replicated on every core.

Sharding: 100000 classes -> 8 cores x 12544 (44 pad rows = -features[0], whose
softmax contribution is ~1e-17 relative). kernel shard is cast to bf16 and
transposed on host (pack only) so DMA streams contiguous at half the bytes and
the PE gets emb-on-partitions.

v4 (from v3's 223-278us baseline; trace showed PE half-clocked by HAM, ACT
square pass + SWDGE flatten dominating):
 - weights stream as bf16: DMA 72->37us, FWL on the bf16 stationary.
 - norm^2 via ones-stationary broadcast matmul accumulating the 4 emb chunks
   of W2 straight into PSUM q-slots: kills the ACT square pass, DVE pairsums,
   the 98 one-row qmm matmuls, and the 7392-packet SWDGE flatten/broadcast.
 - inv = exp(-.5 ln q + ln32) on [128,256] q-chunks; everything ACT ever runs
   (Square/Ln/Exp/Copy) lives in the one natural_log_exp table set. cos(m*pi)
   moved to a DVE sin-polynomial so the trig table never loads.
 - PE is the only saturated engine -> HAM stays at 8/8 after warmup.
 - squares on DVE bf16 (2x mode), v_mult per phase off PSUM, exp+accum on ACT.
"""

import math
import numpy as np

import concourse.bass as bass
import concourse.mybir as mybir
from concourse import library_config
from concourse.bass_utils import run_bass_kernel_spmd

F32 = mybir.dt.float32
BF16 = mybir.dt.bfloat16

# problem constants (hardcoded per harness contract)
B = 256          # batch
E = 512          # embedding
C = 100000       # classes
NCORES = 8
CPC = 12544      # classes per core (padded): 98 groups of 128
TILE_C = 1792    # classes per stream tile
NTILES = CPC // TILE_C             # 7
NPH = 2 * NTILES                   # 14 phases (bc-halves)
QW = 256                           # q broadcast chunk width
NQC = TILE_C // QW                 # 7 q-chunks per tile
CHUNKS = [(0, 512), (512, 512), (1024, 512), (1536, 256)]  # D chunk (off, w)
SCALE = 32.0
M_BASE = 0.5
ALPHA = 0.1
BETA = 0.15
SIN_M = math.sin(M_BASE)
LN32 = math.log(32.0)
ECH = E // 128   # 4 emb chunks

_CACHE = {}


def build_nc():
    nc = bass.Bass(target_bir_lowering=False, num_devices=NCORES, num_swdge_queues=2)

    wt_ext = nc.declare_dram_parameter(
        "wt", [NTILES, ECH, 128, TILE_C], BF16, isOutput=False)
    feat_ext = nc.declare_dram_parameter("feat", [B, E], F32, isOutput=False)
    wlab_ext = nc.declare_dram_parameter("wlab", [B, E], F32, isOutput=False)
    cpack_ext = nc.declare_dram_parameter("cpack", [128, 132], F32, isOutput=False)
    cbf_ext = nc.declare_dram_parameter("cbf", [128, 128], BF16, isOutput=False)
    out_ext = nc.declare_dram_parameter("out", [1, 1], F32, isOutput=True)

    # dummy collective: forces the runtime's synchronized multi-core launch
    cc_in = nc.dram_tensor("cc_in", [1, 1], F32)
    cc_out = nc.dram_tensor("cc_out", [1, NCORES], F32, addr_space="Shared")

    WT_TILE_ELEMS = ECH * 128 * TILE_C

    from contextlib import ExitStack
    ctx = ExitStack()
    sb = lambda name, shape, dt=F32: ctx.enter_context(nc.sbuf_tensor(name, shape, dt))
    ps = lambda name, shape, dt=F32: ctx.enter_context(nc.psum_tensor(name, shape, dt))
    sem = lambda name: ctx.enter_context(nc.semaphore(name))

    with ctx:
        # --- SBUF ---
        WT = [sb(f"WT{i}", [128, ECH, TILE_C], BF16) for i in range(3)]
        W2 = [sb(f"W2{i}", [128, ECH, TILE_C], BF16) for i in range(2)]
        W2H = [sb(f"W2H{i}", [128, 2, TILE_C], BF16) for i in range(2)]
        LNB = [sb(f"LNB{i}", [128, TILE_C]) for i in range(2)]
        INVB = [sb(f"INVB{i}", [128, TILE_C]) for i in range(2)]  # 32/||w||
        CN = [sb(f"CN{i}", [128, TILE_C], BF16) for i in range(2)]  # 32*cos
        EJ = [sb(f"EJ{i}", [128, TILE_C], BF16) for i in range(2)]  # exp junk
        SACC = sb("SACC", [128, 2, NTILES])                     # accum_out slots
        FT = sb("FT", [128, ECH, 2, 128], BF16)                 # fhatT bf16
        F_ = sb("F", [128, 2, E])
        FN = sb("FN", [128, 2, E])
        WL = sb("WL", [128, 2, E])
        WLN = sb("WLN", [128, 2, E])
        SCR = sb("SCR", [128, E])      # square/cosy scratch
        CMP2 = sb("CMP2", [128, B])
        DB = sb("DB", [128, B])        # difficulty broadcast (rank compares)
        DDR = sb("DDR", [1, B])        # difficulty flattened row
        AG = sb("AG", [128, 2 * NCORES])  # remote slot j at cols [2j, 2j+2)
        qfw = sb("qfw", [128, 4])
        lnq4 = sb("lnq4", [128, 4])
        invfw = sb("invfw", [128, 4])
        cosy = sb("cosy", [128, 2]); dd = sb("dd", [128, 2])
        cnt = sb("cnt", [128, 2]); mask = sb("mask", [128, 2])
        t1 = sb("t1", [128, 2]); m015 = sb("m015", [128, 2]); mm_ = sb("mm", [128, 2])
        xs = sb("xs", [128, 2]); x2 = sb("x2", [128, 2])
        pa = sb("pa", [128, 2]); pb = sb("pb", [128, 2]); pc = sb("pc", [128, 2])
        cmpv = sb("cmpv", [128, 2]); t2 = sb("t2", [128, 2]); t3 = sb("t3", [128, 2])
        cosm = sb("cosm", [128, 2]); ey = sb("ey", [128, 2]); em = sb("em", [128, 2])
        adj = sb("adj", [128, 2]); li0c = sb("li0c", [128, 2])
        Sb = sb("Sb", [128, 2]); Sf1 = sb("Sf1", [128, 2])
        Sfull = sb("Sfull", [128, 2]); TT = sb("TT", [128, 2])
        lS = sb("lS", [128, 2]); li = sb("li", [128, 2])
        lsum = sb("lsum", [1, 1]); loss = sb("loss", [1, 1])
        cpack = sb("cpk", [128, 132])
        cbf = sb("cbfs", [128, 128], BF16)

        # --- PSUM: one 16KB tensor, manually laid out ---
        # D ping: chunks {512,512,512,256} at 0/512/1024/1536  [banks 0-3]
        # D pong: 2048 + same                                  [banks 4-7]
        # q(t) broadcast ALSO uses the ping region: PE fills it between
        # v_mult(2t-2) (ping's last reader) and phases(2t) (which waits for
        # ACT's ln chunks to drain it). No dedicated q slots needed.
        # FT staging: pong region [2048:2304) before phases(1)
        # finale loss row: [3840:3842) (bank-7 slack, never used by D/q)
        PS = ps("PS", [128, 4096])
        Doff = lambda ph: (ph % 2) * 2048
        FTOFF = 2048
        FOFF = 3840

        # --- semaphores ---
        s_inF = sem("s_inF"); s_inW = sem("s_inW"); s_cst = sem("s_cst")
        s_wtb = [sem(f"s_wtb{i}") for i in range(3)]
        s_fn = sem("s_fn"); s_wln = sem("s_wln")
        s_sq = sem("s_sq")          # DVE squares done, per tile
        s_w2h = sem("s_w2h")        # GPSIMD ec-pair-sum done, per tile
        s_qc = sem("s_qc")          # PE q-chunk done, global count
        s_lnc = sem("s_lnc")        # ACT ln chunk done, global count
        s_einv = sem("s_einv")      # ACT exp-inv done, per tile
        s_D = sem("s_D"); s_mult = sem("s_mult"); s_exp = sem("s_exp")
        s_ftp = sem("s_ftp"); s_ftc = sem("s_ftc")
        s_cy = sem("s_cy"); s_ddf = sem("s_ddf"); s_db = sem("s_db")
        s_cosm = sem("s_cosm"); s_eyem = sem("s_eyem")
        s_sb = sem("s_sb"); s_tt = sem("s_tt"); s_lns = sem("s_lns")
        s_li = sem("s_li"); s_fin = sem("s_fin"); s_loss = sem("s_loss")
        s_prep = sem("s_prep"); s_rls = sem("s_rls"); s_rs = sem("s_rs")
        s_cc = sem("s_cc")
        s_gd = sem("s_gd")
        s_vh = sem("s_vh"); s_ah = sem("s_ah")

        _hs = {"v": 0, "a": 0}

        def vbar(eng, ins):
            key = "v" if eng.engine == mybir.EngineType.DVE else "a"
            s = s_vh if key == "v" else s_ah
            _hs[key] += 1
            ins.then_inc(s, 1)
            eng.wait_ge(s, _hs[key])

        with nc.Block() as block:

            # ---------------- SYNC: all input DMAs ----------------
            # consts+F first (they gate ACT norms/PE transposes), then WT0,
            # then WL (cosy path, needed ~15us later), then the WT stream.
            @block.sync
            def _(sync):
                sync.dma_start(cpack[:, :], cpack_ext.ap()).then_inc(s_cst, 16)
                sync.dma_start(cbf[:, :], cbf_ext.ap()).then_inc(s_cst, 16)
                for t in range(NTILES):
                    if t >= 3:
                        sync.wait_ge(s_sq, t - 2)        # squares of t-3 done
                        sync.wait_ge(s_D, 2 * (t - 2))   # PE phases of t-3 done
                    sync.dma_start(
                        WT[t % 3][:, :, :],
                        bass.AP(wt_ext, t * WT_TILE_ELEMS,
                                [[TILE_C, 128], [128 * TILE_C, ECH], [1, TILE_C]]),
                    ).then_inc(s_wtb[t % 3], 16)
                    if t == 0:
                        # F right behind WT0 (norms/transposes), WL after
                        sync.dma_start(
                            F_[:, :, :],
                            bass.AP(feat_ext, 0,
                                    [[E, 128], [128 * E, 2], [1, E]]),
                        ).then_inc(s_inF, 16)
                        sync.dma_start(
                            WL[:, :, :],
                            bass.AP(wlab_ext, 0,
                                    [[E, 128], [128 * E, 2], [1, E]]),
                        ).then_inc(s_inW, 16)

            # ---------------- ACT (scalar) ----------------
            @block.scalar
            def _(a):
                Act = mybir.ActivationFunctionType
                neg32 = cpack[:, 129:130]
                ln32c = cpack[:, 130:131]
                # dummy exp on garbage: triggers the one ACT_TABLE_LOAD
                # (natural_log_exp set) while the input DMAs are in flight
                a.activation(SCR[:, 0:1], qfw[:, 0:1], Act.Exp, scale=0.0)
                # f norms first (gates the PE transposes); wlab norms separate
                # so FN doesn't wait on the later WL DMA
                a.wait_ge(s_inF, 16)
                a.activation(SCR[:, :], F_[:, 0, :], Act.Square,
                             accum_out=qfw[:, 0:1])
                ins = a.activation(SCR[:, :], F_[:, 1, :], Act.Square,
                                   accum_out=qfw[:, 1:2])
                vbar(a, ins)
                ins = a.activation(lnq4[:, 0:2], qfw[:, 0:2], Act.Ln)
                vbar(a, ins)
                ins = a.activation(invfw[:, 0:2], lnq4[:, 0:2], Act.Exp,
                                   scale=-0.5)
                vbar(a, ins)
                a.activation(FN[:, 0, :], F_[:, 0, :], Act.Copy,
                             scale=invfw[:, 0:1])
                a.activation(FN[:, 1, :], F_[:, 1, :], Act.Copy,
                             scale=invfw[:, 1:2]).then_inc(s_fn, 1)
                a.wait_ge(s_inW, 16)
                a.activation(SCR[:, :], WL[:, 0, :], Act.Square,
                             accum_out=qfw[:, 2:3])
                ins = a.activation(SCR[:, :], WL[:, 1, :], Act.Square,
                                   accum_out=qfw[:, 3:4])
                vbar(a, ins)
                ins = a.activation(lnq4[:, 2:4], qfw[:, 2:4], Act.Ln)
                vbar(a, ins)
                ins = a.activation(invfw[:, 2:4], lnq4[:, 2:4], Act.Exp,
                                   scale=-0.5)
                vbar(a, ins)
                a.activation(WLN[:, 0, :], WL[:, 0, :], Act.Copy,
                             scale=invfw[:, 2:3])
                a.activation(WLN[:, 1, :], WL[:, 1, :], Act.Copy,
                             scale=invfw[:, 3:4]).then_inc(s_wln, 1)

                def a_lninv(t):
                    # per-tile inv-norm: ln on q chunks (in the ping D region),
                    # then exp(-.5 ln + ln32)
                    for c, (co, cw) in enumerate(CHUNKS):
                        a.wait_ge(s_qc, len(CHUNKS) * t + c + 1)
                        if c == 0 and t >= 2:
                            a.wait_ge(s_einv, t - 1)   # LNB buffer free
                        ins = a.activation(
                            LNB[t % 2][:, co:co + cw],
                            PS[:, co:co + cw],
                            Act.Ln)
                        ins.then_inc(s_lnc, 1)
                    # self-wait on s_lnc = visibility barrier for LNB writes
                    a.wait_ge(s_lnc, len(CHUNKS) * (t + 1))
                    if t >= 2:
                        a.wait_ge(s_mult, 2 * t - 2)   # INVB buffer free
                    a.activation(INVB[t % 2][:, :], LNB[t % 2][:, :], Act.Exp,
                                 bias=ln32c, scale=-0.5).then_inc(s_einv, 1)

                def a_exp(ph):
                    t, half = ph // 2, ph % 2
                    a.wait_ge(s_mult, ph + 1)
                    if ph >= 2:
                        a.wait_ge(s_exp, ph - 1)  # EJ buffer visible-order
                    a.activation(
                        EJ[ph % 2][:, :], CN[ph % 2][:, :], Act.Exp,
                        bias=neg32,
                        accum_out=bass.AP(
                            SACC, half * NTILES + t,
                            [[2 * NTILES, 128], [1, 1]])).then_inc(s_exp, 1)

                a.wait_ge(s_cst, 32)    # ln32c/neg32 resident before first use
                for t in range(NTILES):
                    a_lninv(t)          # lns first: phases(2t) waits on them
                    if t >= 1:          # exps lag one tile, off PE's path
                        a_exp(2 * (t - 1))
                        a_exp(2 * (t - 1) + 1)
                a_exp(2 * (NTILES - 1))
                a_exp(2 * (NTILES - 1) + 1)

                # margin exps: only needed by the finale's adj term
                a.wait_ge(s_cosm, 1)
                a.activation(ey[:, :], cosy[:, :], Act.Exp,
                             bias=neg32, scale=SCALE)
                a.activation(em[:, :], cosm[:, :], Act.Exp,
                             bias=neg32, scale=SCALE).then_inc(s_eyem, 1)

                # finale
                a.wait_ge(s_tt, 1)
                a.activation(lS[:, :], TT[:, :], Act.Ln).then_inc(s_lns, 1)

            # ---------------- GPSIMD: W2 pair-sums, bcasts, exchange, out ---
            @block.gpsimd
            def _(g):
                g.load_library(library_config.proxy)
                # per-tile ec-pair-sum of the squares: W2H = W2[0:2] + W2[2:4]
                # (halves the PE's q-broadcast passes). Runs BEFORE the
                # collective so it can't stall on the slowest core's launch.
                for t in range(NTILES):
                    g.wait_ge(s_sq, t + 1)
                    if t >= 2:
                        g.wait_ge(s_qc, len(CHUNKS) * (t - 1))  # W2H buf free
                    g.tensor_add(
                        bass.AP(W2H[t % 2], 0, [[2 * TILE_C, 128],
                                                [1, 2 * TILE_C]]),
                        bass.AP(W2[t % 2], 0, [[ECH * TILE_C, 128],
                                               [1, 2 * TILE_C]]),
                        bass.AP(W2[t % 2], 2 * TILE_C, [[ECH * TILE_C, 128],
                                                        [1, 2 * TILE_C]]),
                    ).then_inc(s_w2h, 1)
                g.collective_compute(
                    "AllGather", mybir.AluOpType.bypass,
                    replica_groups=[list(range(NCORES))],
                    ins=[cc_in.ap().opt()],
                    outs=[cc_out.ap().opt()],
                ).then_inc(s_cc, 1)
                # difficulty row broadcast for the rank/quantile compares
                g.wait_ge(s_cy, 1)
                g.dma_start(
                    bass.AP(DDR, 0, [[B, 1], [2, 128], [1, 2]]),
                    bass.AP(dd, 0, [[2, 128], [1, 2]]),
                ).then_inc(s_ddf, 16)
                g.wait_ge(s_ddf, 16)
                g.partition_broadcast(DB[:, :], DDR[0:1, :]).then_inc(s_db, 1)
                # remote S-exchange descriptors (addresses only; fired at end).
                for j in range(1, NCORES):
                    g.remote_dma_broadcast(
                        AG[:, 2 * j:2 * j + 2], Sb[:, :],
                        remote_sem=s_rs, local_sem=s_rls,
                        rdests=[(0, j) if k == j else None for k in range(NCORES)],
                        queue_num=1,
                    ).then_inc(s_prep, 1)

                # fire the S exchange once local shard sums are ready
                g.wait_ge(s_cc, 1)
                g.wait_ge(s_prep, NCORES - 1)
                g.wait_ge(s_sb, 1)
                g.trigger_dma(count=NCORES - 1, queue_num=1)
                g.wait_ge(s_loss, 1)
                g.dma_start(out_ext[:, :], loss[:, :]).then_inc(s_gd, 16)
                g.wait_ge(s_gd, 16)

            # ---------------- DVE (vector) ----------------
            @block.vector
            def _(v):
                Alu = mybir.AluOpType

                def v_sq(t):
                    v.wait_ge(s_wtb[t % 3], 16 * (t // 3 + 1))
                    if t >= 2:
                        v.wait_ge(s_w2h, t - 1)   # pair(t-2) done with W2
                    wtf = bass.AP(WT[t % 3], 0, [[ECH * TILE_C, 128],
                                                 [1, ECH * TILE_C]])
                    w2f = bass.AP(W2[t % 2], 0, [[ECH * TILE_C, 128],
                                                 [1, ECH * TILE_C]])
                    v.tensor_mul(w2f, wtf, wtf).then_inc(s_sq, 1)

                def v_mult(ph):
                    t = ph // 2
                    v.wait_ge(s_D, ph + 1)
                    v.wait_ge(s_einv, t + 1)
                    if ph >= 2:
                        v.wait_ge(s_exp, ph - 1)   # CN buffer free
                    v.tensor_mul(
                        CN[ph % 2][:, :],
                        PS[:, Doff(ph):Doff(ph) + TILE_C],
                        INVB[t % 2][:, :]).then_inc(s_mult, 1)

                # tile-0 squares first: nothing upstream of them but the DMA
                v_sq(0)
                # fT chunk copies (ping-pong with PE transposes via PS staging)
                for ec in range(ECH):
                    v.wait_ge(s_ftp, ec + 1)
                    v.tensor_copy(
                        bass.AP(FT, ec * 256, [[ECH * 256, 128], [1, 256]]),
                        PS[:, FTOFF:FTOFF + 256]).then_inc(s_ftc, 1)
                # cos_y (exact fp32) and difficulty
                v.wait_ge(s_wln, 1)
                for b in range(2):
                    ins = v.tensor_mul(SCR[:, :], FN[:, b, :], WLN[:, b, :])
                    vbar(v, ins)
                    ins = v.tensor_reduce(cosy[:, b:b + 1], SCR[:, :],
                                          axis=mybir.AxisListType.X, op=Alu.add)
                    vbar(v, ins)
                v.tensor_scalar(dd[:, :], cosy[:, :], -1.0, 1.0,
                                Alu.mult, Alu.add).then_inc(s_cy, 1)

                # big loop next; the margin chain below waits on the gpsimd
                # difficulty broadcast, which sits behind the launch-sync
                # collective -- putting it before the tile loop would stall
                # the whole pipeline on the slowest core's launch. Squares
                # run one tile ahead, BETWEEN the previous tile's v_mults,
                # so neither q(t) (needs s_sq) nor phases-pong (needs
                # s_mult) ever waits on the DVE queue position.
                v_sq(1)
                for t in range(1, NTILES):
                    v_mult(2 * (t - 1))
                    if t + 1 < NTILES:
                        v_sq(t + 1)
                    v_mult(2 * (t - 1) + 1)
                v_mult(2 * (NTILES - 1))
                v_mult(2 * (NTILES - 1) + 1)

                # rank/quantile: cnt_i = #{j: d_j <= d_i}; mask = cnt >= 52
                v.wait_ge(s_db, 1)
                for b in range(2):
                    ins = v.tensor_scalar(
                        CMP2[:, :], DB[:, :], dd[:, b:b + 1], 0.0,
                        Alu.is_le, Alu.add, accum_out=cnt[:, b:b + 1])
                    vbar(v, ins)
                v.tensor_scalar(mask[:, :], cnt[:, :], 51.5, None, Alu.is_ge)
                ins = v.tensor_scalar(t1[:, :], dd[:, :], ALPHA, M_BASE,
                                      Alu.mult, Alu.add)
                vbar(v, ins)
                ins = v.tensor_scalar(m015[:, :], mask[:, :], BETA, None, Alu.mult)
                vbar(v, ins)
                ins = v.tensor_add(mm_[:, :], t1[:, :], m015[:, :])
                vbar(v, ins)
                # cmpv = -cos(pi*m) = sin(x), x = pi*m - pi/2 in [0, 1.1]:
                # odd Taylor to x^7, |err| < 6e-9 on this range
                ins = v.tensor_scalar(xs[:, :], mm_[:, :], math.pi,
                                      -math.pi / 2.0, Alu.mult, Alu.add)
                vbar(v, ins)
                ins = v.tensor_mul(x2[:, :], xs[:, :], xs[:, :])
                vbar(v, ins)
                ins = v.tensor_scalar(pa[:, :], x2[:, :], -1.0 / 5040.0,
                                      1.0 / 120.0, Alu.mult, Alu.add)
                vbar(v, ins)
                ins = v.tensor_mul(pb[:, :], pa[:, :], x2[:, :])
                vbar(v, ins)
                ins = v.tensor_scalar(pc[:, :], pb[:, :], -1.0 / 6.0, None,
                                      Alu.add)
                vbar(v, ins)
                ins = v.tensor_mul(pa[:, :], pc[:, :], x2[:, :])
                vbar(v, ins)
                ins = v.tensor_scalar(pb[:, :], pa[:, :], 1.0, None, Alu.add)
                vbar(v, ins)
                ins = v.tensor_mul(cmpv[:, :], pb[:, :], xs[:, :])
                vbar(v, ins)
                v.tensor_mul(t2[:, :], cosy[:, :], cmpv[:, :])
                ins = v.tensor_scalar(t3[:, :], mm_[:, :], -SIN_M, None, Alu.mult)
                vbar(v, ins)
                v.tensor_sub(cosm[:, :], t3[:, :], t2[:, :]).then_inc(s_cosm, 1)
                v.wait_ge(s_cosm, 1)  # self-wait doubles as visibility barrier
                v.tensor_scalar(li0c[:, :], cosm[:, :], -SCALE, SCALE,
                                Alu.mult, Alu.add)

                # finale: local shard sum, remote exchange, loss
                v.wait_ge(s_eyem, 1)
                v.tensor_sub(adj[:, :], em[:, :], ey[:, :])
                v.wait_ge(s_exp, NPH)
                ins = v.tensor_reduce(
                    Sb[:, :],
                    bass.AP(SACC, 0, [[2 * NTILES, 128], [NTILES, 2], [1, NTILES]]),
                    axis=mybir.AxisListType.X, op=Alu.add)
                ins.then_inc(s_sb, 1)
                v.wait_ge(s_rs, 2 * (NCORES - 1))   # all 7 peers' slots landed
                ins = v.tensor_reduce(
                    Sf1[:, :],
                    bass.AP(AG, 2, [[2 * NCORES, 128], [1, 2], [2, NCORES - 1]]),
                    axis=mybir.AxisListType.X, op=Alu.add)
                vbar(v, ins)
                ins = v.tensor_add(Sfull[:, :], Sf1[:, :], Sb[:, :])
                vbar(v, ins)
                v.tensor_add(TT[:, :], Sfull[:, :], adj[:, :]).then_inc(s_tt, 1)
                v.wait_ge(s_lns, 1)
                v.tensor_add(li[:, :], lS[:, :], li0c[:, :]).then_inc(s_li, 1)
                v.wait_ge(s_fin, 1)
                ins = v.tensor_reduce(lsum[:, :], PS[0:1, FOFF:FOFF + 2],
                                      axis=mybir.AxisListType.X, op=Alu.add)
                vbar(v, ins)
                v.tensor_scalar(loss[:, :], lsum[:, :], 1.0 / B, None,
                                Alu.mult).then_inc(s_loss, 1)

            # ---------------- PE (tensor) ----------------
            @block.tensor
            def _(te):
                te.wait_ge(s_cst, 32)
                te.wait_ge(s_fn, 1)
                identf = cpack[:, 0:128]   # fp32 identity for the transposes
                onesf = cpack[:, 128:129]  # fp32 ones column (finale mean)
                onesq = cbf[:, 0:128]      # bf16 all-ones 128x128 (norm bcast)
                # fT = transpose(f_norm): [e_p, ec, bc, b] via PS staging
                for ec in range(ECH):
                    if ec >= 1:
                        te.wait_ge(s_ftc, ec)
                    te.transpose(PS[:, FTOFF:FTOFF + 128],
                                 FN[:, 0, ec * 128:(ec + 1) * 128], identf)
                    te.transpose(PS[:, FTOFF + 128:FTOFF + 256],
                                 FN[:, 1, ec * 128:(ec + 1) * 128],
                                 identf).then_inc(s_ftp, 1)

                def t_qmm(t):
                    # norm^2 broadcast: ones^T @ W2H chunks (2 passes),
                    # written into the ping D region (free after
                    # v_mult(2t-2); phases(2t) waits for ACT's lns to drain it)
                    te.wait_ge(s_w2h, t + 1)
                    if t >= 1:
                        te.wait_ge(s_mult, 2 * t - 1)   # ping region free
                    for c, (co, cw) in enumerate(CHUNKS):
                        for j in range(2):
                            ins = te.matmul(
                                PS[:, co:co + cw],
                                onesq,
                                W2H[t % 2][:, j, co:co + cw],
                                start=(j == 0), stop=(j == 1),
                                skip_group_check=True)
                        ins.then_inc(s_qc, 1)

                def t_phase(ph):
                    t, half = ph // 2, ph % 2
                    if half == 0:
                        te.wait_ge(s_wtb[t % 3], 16 * (t // 3 + 1))
                        # ping region: ACT lns of q(t) must have drained it
                        te.wait_ge(s_lnc, len(CHUNKS) * (t + 1))
                    else:
                        if ph >= 2:
                            te.wait_ge(s_mult, ph - 1)  # pong D free
                    if ph == 1:
                        te.wait_ge(s_ftc, ECH)          # FT staging drained
                    for (co, cw) in CHUNKS:
                        for ec in range(ECH):
                            ins = te.matmul(
                                PS[:, Doff(ph) + co:Doff(ph) + co + cw],
                                FT[:, ec, half, :],
                                WT[t % 3][:, ec, co:co + cw],
                                start=(ec == 0), stop=(ec == ECH - 1),
                                skip_group_check=True)
                    ins.then_inc(s_D, 1)

                for t in range(NTILES):
                    t_qmm(t)
                    t_phase(2 * t)
                    t_phase(2 * t + 1)

                # finale: batch mean via ones-stationary matmul into q slack
                te.wait_ge(s_li, 1)
                te.matmul(PS[0:1, FOFF:FOFF + 2], onesf, li[:, :]).then_inc(s_fin, 1)

        return nc


def _shard_host(features, labels, kernel_w):
    """Host-side shard + pack (layout + bf16 cast only, no arithmetic)."""
    import ml_dtypes
    features = np.ascontiguousarray(features, dtype=np.float32)
    kernel_w = np.ascontiguousarray(kernel_w, dtype=np.float32)
    labels = np.asarray(labels).astype(np.int64)
    wlab = np.ascontiguousarray(kernel_w[labels])        # (B, E) gather
    pad_row = -features[0]                               # direction only matters
    cpack = np.zeros((128, 132), dtype=np.float32)
    cpack[:, 0:128] = np.eye(128, dtype=np.float32)
    cpack[:, 128] = 1.0          # onesf
    cpack[:, 129] = -SCALE       # neg32
    cpack[:, 130] = LN32         # ln32c
    cbf = np.ones((128, 128), dtype=np.float32).astype(ml_dtypes.bfloat16)
    in_maps = []
    cpc_raw = C // NCORES                                # 12500
    for c in range(NCORES):
        shard = kernel_w[c * cpc_raw:(c + 1) * cpc_raw]  # (12500, E)
        pad = np.broadcast_to(pad_row, (CPC - cpc_raw, E))
        shard = np.concatenate([shard, pad], axis=0)     # (12544, E)
        # (CPC, E) -> transpose -> (E, CPC) -> (ECH,128, NTILES,TILE_C)
        wt = shard.T.reshape(ECH, 128, NTILES, TILE_C)
        wt = np.ascontiguousarray(wt.transpose(2, 0, 1, 3)).astype(ml_dtypes.bfloat16)
        in_maps.append({"wt": wt, "feat": features, "wlab": wlab,
                        "cpack": cpack, "cbf": cbf})
    return in_maps


def _get_nc():
    if "nc" not in _CACHE:
        nc = build_nc()
        from concourse.library_overlay import lower_extended_insts
        lower_extended_insts(nc)
        _CACHE["nc"] = nc
    return _CACHE["nc"]


def kernel(features, labels, kernel):
    in_maps = _shard_host(features, labels, kernel)
    nc = _get_nc()
    res = run_bass_kernel_spmd(nc, in_maps, core_ids=list(range(NCORES)))
    out = res.results[0]["out"]
    return np.float32(out.reshape(())[()])
